# revision 2
# baseline (speedup 1.0000x reference)
"""Trainium2 Bass kernel for nn_Block_52278341927299 (dense transformer block).

Sharding: H-dim split 8 ways (2 rows of 16 per core -> 1024 contiguous
spatial positions each). MLP + qkv fully local; k/v AllGathered across
cores; each core runs attention for its 1024 queries over all 8192 keys.

Instruction-count-optimized rewrite:
- plain fp32 matmuls everywhere (self-loading weights, no Ldweights)
- q/k computed directly in [d, l] layout (no PE transposes); per-position
  norms via gpsimd partition_all_reduce
- x-norm via partition_all_reduce (no ones/broadcast matmuls)
- softmax denominators via strided middle-axis tensor_reduce +
  partition_all_reduce (no ones-matmul accumulation)
- depthwise taps read a single zero-padded copy at odd offsets
"""
import sys

if '/opt/trn_rl_repo' not in sys.path:
    sys.path.insert(0, '/opt/trn_rl_repo')

import numpy as np

import concourse.bass as bass
import concourse.bacc as bacc
import concourse.bass_isa as bass_isa
import concourse.mybir as mybir
import concourse.tile as tile
from concourse.bass_utils import run_bass_kernel_spmd

F32 = mybir.dt.float32
AF = mybir.ActivationFunctionType
RADD = bass_isa.ReduceOp.add

R = 8            # cores
C = 512          # channels
CT = 4           # channel tiles of 128
M = 1024         # mlp hidden
MT = 8
HEADS = 4
D = 128          # head dim
Lc = 1024        # local positions per core (2 rows x 512)
L = 8192         # total positions
KT = 64          # key tiles of 128
EPS = 1e-4
SILU_SCALE = 0.596
CLIP = 256.0
ISQ_D = 1.0 / np.sqrt(128.0)   # exp scale (1/sqrt(d))
ISQ2 = 1.0 / np.sqrt(2.0)
SQ2 = float(np.sqrt(2.0))

_CACHE = {}


def _build_nc(reps=1):
    nc = bacc.Bacc(num_devices=R)

    # ---------------- I/O ----------------
    g = {}
    g["xs_d"] = nc.declare_dram_parameter("xs", [C, Lc], F32, isOutput=False)
    g["pos_d"] = nc.declare_dram_parameter("pos", [C, Lc], F32, isOutput=False)
    g["emb_d"] = nc.declare_dram_parameter("embv", [C], F32, isOutput=False)
    g["w0t_d"] = nc.declare_dram_parameter("w0t", [C, M], F32, isOutput=False)
    g["wdt_d"] = nc.declare_dram_parameter("wdt", [128, 72 * 128], F32,
                                           isOutput=False)
    g["wembt_d"] = nc.declare_dram_parameter("wembt", [C, M], F32,
                                             isOutput=False)
    g["w1t_d"] = nc.declare_dram_parameter("w1t", [M, C], F32, isOutput=False)
    g["wq_d"] = nc.declare_dram_parameter("wq", [C, 2, C], F32, isOutput=False)
    g["wk_d"] = nc.declare_dram_parameter("wk", [C, 2, C], F32, isOutput=False)
    g["wvt_d"] = nc.declare_dram_parameter("wvt", [C, C], F32, isOutput=False)
    g["out_d"] = nc.declare_dram_parameter("out", [C, Lc], F32, isOutput=True)

    # internal DRAM
    g["cdram"] = nc.dram_tensor("cdram", [1, M], F32)
    g["agk_in"] = nc.dram_tensor("agk_in", [C, Lc], F32)
    g["agv_in"] = nc.dram_tensor("agv_in", [Lc, C], F32)
    g["agk_out"] = nc.dram_tensor("agk_out", [R, HEADS, 128, Lc], F32,
                                  addr_space="Shared")
    g["agv_out"] = nc.dram_tensor("agv_out", [KT, 128, C], F32,
                                  addr_space="Shared")

    with tile.TileContext(nc) as tc:
        with tc.tile_pool(name="persist", bufs=1) as pp:
            qn = pp.tile([128, HEADS, Lc], F32)  # normalized q, [d, l]
            xh = pp.tile([128, CT, Lc], F32)     # xm/2, then += attn
            c_col = pp.tile([128, MT], F32)      # emb modulation
            g["qn"], g["xh"], g["c_col"] = qn, xh, c_col
            for _rep in range(reps):
                _build_body(nc, tc, pp, g)
    nc.compile()
    return nc


def _build_body(nc, tc, pp, g):
    qn = g["qn"]; xh = g["xh"]; c_col = g["c_col"]

    with tc.tile_pool(name="pmid", bufs=1) as pmid:
        xm = pmid.tile([128, CT, Lc], F32)   # sqrt(2) * x_mid
        _build_front(nc, tc, pp, g, xm, qn, xh, c_col)
    _build_attention(nc, tc, pp, g, qn, xh)


def _build_front(nc, tc, pp, g, xm, qn, xh, c_col):
    with tc.tile_pool(name="pAB", bufs=1) as pAB:
        xn = pAB.tile([128, CT, Lc], F32)
        y2 = pAB.tile([128, MT, Lc], F32)

        # ============ emb modulation c (PE-light: 8 matmuls) ============
        with tc.tile_pool(name="p0", bufs=1) as p0, \
             tc.tile_pool(name="p0ps", bufs=1, space="PSUM") as p0ps:
            wembt_sb = p0.tile([128, CT, M], F32)
            nc.sync.dma_start(
                wembt_sb[:],
                g["wembt_d"][:].rearrange("(t p) co -> p t co", p=128))
            emb_sb = p0.tile([128, CT], F32)
            nc.sync.dma_start(
                emb_sb[:], g["emb_d"][:].rearrange("(t p) -> p t", p=128))
            crow_ps = p0ps.tile([1, M], F32)
            for co2 in range(2):
                for t in range(CT):
                    nc.tensor.matmul(
                        crow_ps[0:1, co2 * 512:(co2 + 1) * 512],
                        emb_sb[:, t:t + 1],
                        wembt_sb[:, t, co2 * 512:(co2 + 1) * 512],
                        start=(t == 0), stop=(t == CT - 1))
            c_row = p0.tile([1, M], F32)
            nc.scalar.add(c_row[:], crow_ps[:], 1.0)
            nc.sync.dma_start(g["cdram"][:], c_row[:])
            nc.sync.dma_start(
                c_col[:],
                g["cdram"][:].rearrange("o (gg p) -> (o p) gg", p=128))

        # ============ x-norm ============
        with tc.tile_pool(name="p1", bufs=1) as p1:
            xs = p1.tile([128, CT, Lc], F32)
            nc.sync.dma_start(
                xs[:], g["xs_d"][:].rearrange("(t p) l -> p t l", p=128))
            sq = p1.tile([128, CT, Lc], F32)
            nc.vector.tensor_mul(sq[:], xs[:], xs[:])
            s01 = p1.tile([128, Lc], F32)
            nc.vector.tensor_add(s01[:], sq[:, 0, :], sq[:, 1, :])
            s23 = p1.tile([128, Lc], F32)
            nc.vector.tensor_add(s23[:], sq[:, 2, :], sq[:, 3, :])
            s_all = p1.tile([128, Lc], F32)
            nc.vector.tensor_add(s_all[:], s01[:], s23[:])
            ssq = p1.tile([128, Lc], F32)
            nc.gpsimd.partition_all_reduce(ssq[:], s_all[:], channels=128,
                                           reduce_op=RADD)
            nrm = p1.tile([128, Lc], F32)
            nc.scalar.activation(nrm[:], ssq[:], AF.Sqrt, scale=1.0 / C)
            nc.vector.tensor_scalar_add(nrm[:], nrm[:], EPS)
            inv = p1.tile([128, Lc], F32)
            nc.vector.reciprocal_approx_fast(inv[:], nrm[:])
            nc.vector.tensor_mul(
                xn[:], xs[:],
                inv[:, None, :].broadcast_to([128, CT, Lc]))

        # ============ MLP: res0 + depth conv + silu ============
        with tc.tile_pool(name="p2a", bufs=1) as p2a, \
             tc.tile_pool(name="p2aps", bufs=1, space="PSUM") as p2aps:
            w0t_sb = p2a.tile([128, CT, M], F32)
            nc.sync.dma_start(
                w0t_sb[:],
                g["w0t_d"][:].rearrange("(t p) co -> p t co", p=128))
            wdt_sb = p2a.tile([128, 72, 128], F32)
            nc.sync.dma_start(
                wdt_sb[:],
                g["wdt_d"][:].rearrange("p (s co) -> p s co", co=128))
            for gi in range(MT):
                y0ps = p2aps.tile([128, Lc], F32, tag="y0ps", bufs=2)
                for lc in range(2):
                    for t in range(CT):
                        nc.tensor.matmul(
                            y0ps[:, lc * 512:(lc + 1) * 512],
                            w0t_sb[:, t, gi * 128:(gi + 1) * 128],
                            xn[:, t, lc * 512:(lc + 1) * 512],
                            start=(t == 0), stop=(t == CT - 1))
                y0p = p2a.tile([128, 2, 520], F32, tag="y0p", bufs=2)
                if gi < 2:
                    # zero the halos once per rotating buffer
                    nc.any.memset(y0p[:, :, 0:4], 0.0)
                    nc.any.memset(y0p[:, :, 516:520], 0.0)
                for row in range(2):
                    nc.vector.tensor_copy(
                        y0p[:, row, 4:516],
                        y0ps[:, row * 512:(row + 1) * 512])
                y1ps = p2aps.tile([128, Lc], F32, tag="y1ps", bufs=2)
                for row in range(2):
                    for tap in range(9):
                        nc.tensor.matmul(
                            y1ps[:, row * 512:(row + 1) * 512],
                            wdt_sb[:, gi * 9 + tap, :],
                            y0p[:, row, tap:tap + 512],
                            start=(tap == 0), stop=(tap == 8))
                nc.scalar.activation(
                    y2[:, gi, :], y1ps[:], AF.Silu,
                    scale=c_col[:, gi:gi + 1])

        # ---- res1 + x_mid ----
        with tc.tile_pool(name="p2b", bufs=1) as p2b, \
             tc.tile_pool(name="p2bps", bufs=1, space="PSUM") as p2bps:
            w1t_sb = p2b.tile([128, MT, C], F32)
            nc.sync.dma_start(
                w1t_sb[:],
                g["w1t_d"][:].rearrange("(gg p) co -> p gg co", p=128))
            for lc in range(2):
                for co in range(CT):
                    y3ps = p2bps.tile([128, 512], F32, tag="y3ps", bufs=4)
                    for gi in range(MT):
                        nc.tensor.matmul(
                            y3ps[:],
                            w1t_sb[:, gi, co * 128:(co + 1) * 128],
                            y2[:, gi, lc * 512:(lc + 1) * 512],
                            start=(gi == 0), stop=(gi == MT - 1))
                    nc.vector.tensor_add(
                        xm[:, co, lc * 512:(lc + 1) * 512],
                        xn[:, co, lc * 512:(lc + 1) * 512],
                        y3ps[:])

    # ============ qkv ============
    with tc.tile_pool(name="p3", bufs=1) as p3, \
         tc.tile_pool(name="p3ps", bufs=1, space="PSUM") as p3ps:
        wk_sb = p3.tile([128, CT, 2, C], F32)
        nc.sync.dma_start(
            wk_sb[:], g["wk_d"][:].rearrange("(t p) e co -> p t e co", p=128))
        wq_sb = p3.tile([128, CT, 2, C], F32)
        nc.sync.dma_start(
            wq_sb[:], g["wq_d"][:].rearrange("(t p) e co -> p t e co", p=128))
        wvt_sb = p3.tile([128, CT, C], F32)
        nc.sync.dma_start(
            wvt_sb[:], g["wvt_d"][:].rearrange("(t p) co -> p t co", p=128))
        xpos = p3.tile([128, CT, Lc], F32)
        with tc.tile_pool(name="p3pos", bufs=1) as p3pos:
            pos = p3pos.tile([128, CT, Lc], F32)
            nc.sync.dma_start(
                pos[:], g["pos_d"][:].rearrange("(t p) l -> p t l", p=128))
            nc.vector.tensor_mul(xpos[:], xm[:], pos[:])

        def qk_proj(w_sb, raw, nrm):
            for h in range(HEADS):
                for lc in range(2):
                    ps = p3ps.tile([128, 512], F32, tag="qkps", bufs=4)
                    for t in range(CT):
                        nc.tensor.matmul(
                            ps[:], w_sb[:, t, 0, h * 128:(h + 1) * 128],
                            xm[:, t, lc * 512:(lc + 1) * 512],
                            start=(t == 0), stop=False)
                    for t in range(CT):
                        nc.tensor.matmul(
                            ps[:], w_sb[:, t, 1, h * 128:(h + 1) * 128],
                            xpos[:, t, lc * 512:(lc + 1) * 512],
                            start=False, stop=(t == CT - 1))
                    sqt = p3.tile([128, 512], F32, tag="sqt", bufs=2)
                    nc.scalar.square(sqt[:], ps[:])
                    nc.gpsimd.partition_all_reduce(
                        nrm[:, h, lc * 512:(lc + 1) * 512], sqt[:],
                        channels=128, reduce_op=RADD)
                    nc.vector.tensor_copy(
                        raw[:, h, lc * 512:(lc + 1) * 512], ps[:])

        def finish_norm(raw, nrm, invt):
            nc.scalar.activation(invt[:], nrm[:], AF.Sqrt, scale=1.0 / D)
            nc.vector.tensor_scalar_add(invt[:], invt[:], EPS)
            nc.vector.reciprocal_approx_fast(nrm[:], invt[:])
            nc.vector.tensor_mul(raw[:], raw[:], nrm[:])

        # ---- k, then AllGather k ----
        with tc.tile_pool(name="p3k", bufs=1) as p3k:
            kn = p3k.tile([128, HEADS, Lc], F32)
            nk = p3k.tile([128, HEADS, Lc], F32)
            ik = p3k.tile([128, HEADS, Lc], F32)
            qk_proj(wk_sb, kn, nk)
            finish_norm(kn, nk, ik)
            nc.sync.dma_start(
                g["agk_in"][:].rearrange("(h p) l -> p h l", p=128), kn[:])
            nc.gpsimd.collective_compute(
                "AllGather", mybir.AluOpType.bypass,
                replica_groups=[list(range(R))],
                ins=[g["agk_in"][:]], outs=[g["agk_out"][:]])

        # ---- v, then AllGather v ----
        with tc.tile_pool(name="p3v", bufs=1) as p3v:
            vraw = p3v.tile([128, 8, C], F32)
            nvs = p3v.tile([128, 8, HEADS], F32)
            ivv = p3v.tile([128, 8, HEADS], F32)
            for lt in range(8):
                vps = p3ps.tile([128, 512], F32, tag="qkps", bufs=4)
                for t in range(CT):
                    nc.tensor.matmul(
                        vps[:], xm[:, t, lt * 128:(lt + 1) * 128],
                        wvt_sb[:, t, :],
                        start=(t == 0), stop=(t == CT - 1))
                sqt = p3.tile([128, 512], F32, tag="sqt", bufs=2)
                nc.scalar.square(sqt[:], vps[:])
                nc.vector.tensor_reduce(
                    nvs[:, lt, :],
                    sqt[:].rearrange("p (h d) -> p h d", d=128),
                    axis=mybir.AxisListType.X, op=mybir.AluOpType.add)
                nc.vector.tensor_copy(vraw[:, lt, :], vps[:])
            nc.scalar.activation(ivv[:], nvs[:], AF.Sqrt, scale=1.0 / D)
            nc.vector.tensor_scalar_add(ivv[:], ivv[:], EPS)
            nc.vector.reciprocal_approx_fast(nvs[:], ivv[:])
            for lt in range(8):
                vv = vraw[:, lt, :].rearrange("p (h d) -> p h d", d=128)
                nc.vector.tensor_mul(
                    vv, vv,
                    nvs[:, lt, :, None].broadcast_to([128, HEADS, 128]))
            nc.sync.dma_start(
                g["agv_in"][:].rearrange("(lt p) co -> p lt co", p=128),
                vraw[:])
            nc.gpsimd.collective_compute(
                "AllGather", mybir.AluOpType.bypass,
                replica_groups=[list(range(R))],
                ins=[g["agv_in"][:]], outs=[g["agv_out"][:]])

        # ---- q (overlaps the collectives) ----
        with tc.tile_pool(name="p3q", bufs=1) as p3q:
            nq = p3q.tile([128, HEADS, Lc], F32)
            iq = p3q.tile([128, HEADS, Lc], F32)
            qk_proj(wq_sb, qn, nq)
            finish_norm(qn, nq, iq)
        nc.vector.tensor_scalar_mul(xh[:], xm[:], 0.5)


def _build_attention(nc, tc, pp, g, qn, xh):
    with tc.tile_pool(name="p4", bufs=1) as p4, \
         tc.tile_pool(name="p4ps", bufs=1, space="PSUM") as p4ps:
        NB = 8          # exp batches per quarter (2 ktiles per batch)
        NQ = 4          # quarters of the key range (16 ktiles each)
        for h in range(HEADS):
            ksb = p4.tile([128, KT * 128], F32, tag="ksb", bufs=2)
            nc.sync.dma_start(
                ksb[:].rearrange("p (r l) -> p r l", r=R),
                g["agk_out"][:, h, :, :].rearrange("r p l -> p r l"))
            vsb = p4.tile([128, KT, 128], F32, tag="vsb", bufs=1)
            nc.sync.dma_start(
                vsb[:],
                g["agv_out"][:, :, h * 128:(h + 1) * 128]
                .rearrange("t p d -> p t d"))
            for cch in range(2):
                q_c = qn[:, h, cch * 512:(cch + 1) * 512]
                pvps = p4ps.tile([128, 512], F32, tag="pvps", bufs=2)
                parts = p4.tile([128, NQ, Lc], F32, tag="parts", bufs=1)
                for qu in range(NQ):
                    e_t = p4.tile([128, NB, Lc], F32, tag="e", bufs=1)
                    for b in range(NB):
                        t0 = qu * 16 + b * 2
                        sps = p4ps.tile([128, 1024], F32, tag="sps", bufs=2)
                        for j in range(2):
                            nc.tensor.matmul(
                                sps[:, j * 512:(j + 1) * 512],
                                ksb[:, (t0 + j) * 128:(t0 + j + 1) * 128],
                                q_c,
                                start=True, stop=True)
                        nc.scalar.activation(
                            e_t[:, b, :], sps[:], AF.Exp, scale=ISQ_D)
                        for j in range(2):
                            t = t0 + j
                            nc.tensor.matmul(
                                pvps[:],
                                vsb[:, t, :],
                                e_t[:, b, j * 512:(j + 1) * 512],
                                start=(t == 0), stop=(t == KT - 1))
                    nc.vector.tensor_reduce(
                        parts[:, qu, :],
                        e_t[:].rearrange("p b l -> p l b"),
                        axis=mybir.AxisListType.X, op=mybir.AluOpType.add)
                s01 = p4.tile([128, Lc], F32, tag="s01", bufs=1)
                nc.vector.tensor_add(s01[:], parts[:, 0, :], parts[:, 1, :])
                s23 = p4.tile([128, Lc], F32, tag="s23", bufs=1)
                nc.vector.tensor_add(s23[:], parts[:, 2, :], parts[:, 3, :])
                nc.vector.tensor_add(s01[:], s01[:], s23[:])
                # fold the two ktile-parity halves; scale by sqrt(2)
                ssum = p4.tile([128, 512], F32, tag="ssum", bufs=1)
                nc.vector.tensor_add(ssum[:], s01[:, 0:512], s01[:, 512:1024])
                sred = p4.tile([128, 512], F32, tag="sred", bufs=1)
                nc.gpsimd.partition_all_reduce(sred[:], ssum[:], channels=128,
                                               reduce_op=RADD)
                nc.vector.tensor_scalar_mul(sred[:], sred[:], SQ2)
                isum = p4.tile([128, 512], F32, tag="isum", bufs=1)
                nc.vector.reciprocal_approx_fast(isum[:], sred[:])
                att = p4.tile([128, 512], F32, tag="att", bufs=2)
                nc.vector.tensor_mul(att[:], pvps[:], isum[:])
                nc.vector.tensor_add(
                    xh[:, h, cch * 512:(cch + 1) * 512],
                    xh[:, h, cch * 512:(cch + 1) * 512],
                    att[:])
        nc.vector.tensor_scalar(
            xh[:], xh[:], CLIP, -CLIP,
            op0=mybir.AluOpType.min, op1=mybir.AluOpType.max)
        nc.sync.dma_start(
            g["out_d"][:].rearrange("(t p) l -> p t l", p=128), xh[:])


def _host_prep(x, emb, pos_emb, emb_gain, w_res0, w_depth, w_emb, w_res1,
               w_qk, w_v):
    """Build shared weight arrays + per-core input shards."""
    f = np.float32
    w_res0 = np.asarray(w_res0, f).reshape(M, C)
    w_depth = np.asarray(w_depth, f).reshape(M, 128, 9)
    w_emb = np.asarray(w_emb, f).reshape(M, C)
    w_res1 = np.asarray(w_res1, f).reshape(C, M)
    w_qk = np.asarray(w_qk, f).reshape(2 * C, 2 * C)
    w_v = np.asarray(w_v, f).reshape(C, C)
    emb_gain = np.float32(emb_gain)

    w0t = np.ascontiguousarray((w_res0 * (1.0 / np.sqrt(C))).T)     # [C, M]
    wd = w_depth * (1.0 / np.sqrt(128 * 9))
    # wdt[p, (g*9+t)*128 + co] = wd[g*128+co, p, t]
    wdt = np.empty((128, 72 * 128), f)
    for gi in range(MT):
        blk = wd[gi * 128:(gi + 1) * 128]        # [co=128, ci=128, tap=9]
        wdt[:, gi * 9 * 128:(gi + 1) * 9 * 128] = (
            blk.transpose(1, 2, 0).reshape(128, 9 * 128))
    wembt = np.ascontiguousarray((w_emb * (emb_gain / np.sqrt(C))).T)  # [C, M]
    w1t = np.ascontiguousarray(
        (w_res1 * (1.0 / (SILU_SCALE * np.sqrt(M)))).T)             # [M, C]

    # qk rows: r = h*256 + dd*2 + s; cols: 2c (x part), 2c+1 (x*pos part)
    wqk = w_qk * (1.0 / np.sqrt(2 * C))
    wq3 = wqk.reshape(HEADS, 128, 2, 2 * C)   # [h, dd, s, ci2]
    wq_rows = wq3[:, :, 0, :].reshape(C, 2 * C)   # q rows [(h dd), ci2]
    wk_rows = wq3[:, :, 1, :].reshape(C, 2 * C)
    wq = np.empty((C, 2, C), f)
    wq[:, 0, :] = (wq_rows[:, 0::2] * ISQ2).T     # even ci: xm part
    wq[:, 1, :] = wq_rows[:, 1::2].T              # odd ci: xm*pos part
    wk = np.empty((C, 2, C), f)
    wk[:, 0, :] = (wk_rows[:, 0::2] * ISQ2).T
    wk[:, 1, :] = wk_rows[:, 1::2].T
    wvt = np.ascontiguousarray((w_v * (1.0 / np.sqrt(C) * ISQ2)).T)  # [C, C]

    x = np.asarray(x, f).reshape(C, 16, 512)
    pos = np.asarray(pos_emb, f).reshape(C, 16, 512) * ISQ2
    embv = np.ascontiguousarray(np.asarray(emb, f).reshape(C))

    shared = dict(
        embv=embv, w0t=w0t, wdt=wdt, wembt=wembt, w1t=w1t,
        wq=wq, wk=wk, wvt=wvt)
    in_maps = []
    for r in range(R):
        m = dict(shared)
        m["xs"] = np.ascontiguousarray(
            x[:, 2 * r:2 * r + 2, :].reshape(C, Lc))
        m["pos"] = np.ascontiguousarray(
            pos[:, 2 * r:2 * r + 2, :].reshape(C, Lc))
        in_maps.append(m)
    return in_maps


def kernel(**inputs):
    if "nc" not in _CACHE:
        _CACHE["nc"] = _build_nc()
    nc = _CACHE["nc"]
    in_maps = _host_prep(
        inputs["x"], inputs["emb"], inputs["pos_emb"], inputs["emb_gain"],
        inputs["w_res0"], inputs["w_depth"], inputs["w_emb"],
        inputs["w_res1"], inputs["w_qk"], inputs["w_v"])
    res = run_bass_kernel_spmd(nc, in_maps, list(range(R)))
    out = np.empty((1, C, 16, 512), np.float32)
    for r in range(R):
        out[0, :, 2 * r:2 * r + 2, :] = res.results[r]["out"].reshape(C, 2, 512)
    return out


# revision 4
# speedup vs baseline: 2.4187x; 2.4187x over previous
"""Trainium2 Bass kernel for nn_Block_52278341927299 (dense transformer block).

Sharding: H-dim split 8 ways (2 rows of 16 per core -> 1024 contiguous
spatial positions each). MLP + qkv fully local; k/v AllGathered across
cores; each core runs attention for its 1024 queries over all 8192 keys.

Instruction-count-optimized rewrite:
- plain fp32 matmuls everywhere (self-loading weights, no Ldweights)
- q/k computed directly in [d, l] layout (no PE transposes); per-position
  norms via gpsimd partition_all_reduce
- x-norm via partition_all_reduce (no ones/broadcast matmuls)
- softmax denominators via strided middle-axis tensor_reduce +
  partition_all_reduce (no ones-matmul accumulation)
- depthwise taps read a single zero-padded copy at odd offsets
"""
import sys

if '/opt/trn_rl_repo' not in sys.path:
    sys.path.insert(0, '/opt/trn_rl_repo')

import numpy as np

import concourse.bass as bass
import concourse.bacc as bacc
import concourse.bass_isa as bass_isa
import concourse.mybir as mybir
import concourse.tile as tile
from concourse.bass_utils import run_bass_kernel_spmd

F32 = mybir.dt.float32
AF = mybir.ActivationFunctionType
RADD = bass_isa.ReduceOp.add

R = 8            # cores
C = 512          # channels
CT = 4           # channel tiles of 128
M = 1024         # mlp hidden
MT = 8
HEADS = 4
D = 128          # head dim
Lc = 1024        # local positions per core (2 rows x 512)
L = 8192         # total positions
KT = 64          # key tiles of 128
EPS = 1e-4
SILU_SCALE = 0.596
CLIP = 256.0
ISQ_D = 1.0 / np.sqrt(128.0)   # exp scale (1/sqrt(d))
ISQ2 = 1.0 / np.sqrt(2.0)
SQ2 = float(np.sqrt(2.0))

_CACHE = {}


def _build_nc(reps=1):
    nc = bacc.Bacc(num_devices=R)

    # ---------------- I/O ----------------
    g = {}
    g["xs_d"] = nc.declare_dram_parameter("xs", [C, Lc], F32, isOutput=False)
    g["pos_d"] = nc.declare_dram_parameter("pos", [C, Lc], F32, isOutput=False)
    g["emb_d"] = nc.declare_dram_parameter("embv", [C], F32, isOutput=False)
    g["w0t_d"] = nc.declare_dram_parameter("w0t", [C, M], F32, isOutput=False)
    g["wdt_d"] = nc.declare_dram_parameter("wdt", [128, 72 * 128], F32,
                                           isOutput=False)
    g["wembt_d"] = nc.declare_dram_parameter("wembt", [C, M], F32,
                                             isOutput=False)
    g["w1t_d"] = nc.declare_dram_parameter("w1t", [M, C], F32, isOutput=False)
    g["wq_d"] = nc.declare_dram_parameter("wq", [C, 2, C], F32, isOutput=False)
    g["wk_d"] = nc.declare_dram_parameter("wk", [C, 2, C], F32, isOutput=False)
    g["wvt_d"] = nc.declare_dram_parameter("wvt", [C, C], F32, isOutput=False)
    g["out_d"] = nc.declare_dram_parameter("out", [C, Lc], F32, isOutput=True)

    # internal DRAM
    g["cdram"] = nc.dram_tensor("cdram", [1, M], F32)
    g["agk_in"] = nc.dram_tensor("agk_in", [C, Lc], F32)
    g["agv_in"] = nc.dram_tensor("agv_in", [Lc, C], F32)
    g["agk_out"] = nc.dram_tensor("agk_out", [R, HEADS, 128, Lc], F32,
                                  addr_space="Shared")
    g["agv_out"] = nc.dram_tensor("agv_out", [KT, 128, C], F32,
                                  addr_space="Shared")

    with tile.TileContext(nc) as tc:
        with tc.tile_pool(name="persist", bufs=1) as pp:
            qn = pp.tile([128, HEADS, Lc], F32)  # normalized q, [d, l]
            xh = pp.tile([128, CT, Lc], F32)     # xm/2, then += attn
            c_col = pp.tile([128, MT], F32)      # emb modulation
            g["qn"], g["xh"], g["c_col"] = qn, xh, c_col
            for _rep in range(reps):
                _build_body(nc, tc, pp, g)
    nc.compile()
    return nc


def _build_body(nc, tc, pp, g):
    qn = g["qn"]; xh = g["xh"]; c_col = g["c_col"]

    with tc.tile_pool(name="pmid", bufs=1) as pmid:
        xm = pmid.tile([128, CT, Lc], F32)   # sqrt(2) * x_mid
        _build_front(nc, tc, pp, g, xm, qn, xh, c_col)
    _build_attention(nc, tc, pp, g, qn, xh)


def _build_front(nc, tc, pp, g, xm, qn, xh, c_col):
    with tc.tile_pool(name="pAB", bufs=1) as pAB:
        xn = pAB.tile([128, CT, Lc], F32)
        y2 = pAB.tile([128, MT, Lc], F32)

        # ============ emb modulation c (PE-light: 8 matmuls) ============
        with tc.tile_pool(name="p0", bufs=1) as p0, \
             tc.tile_pool(name="p0ps", bufs=1, space="PSUM") as p0ps:
            wembt_sb = p0.tile([128, CT, M], F32)
            nc.sync.dma_start(
                wembt_sb[:],
                g["wembt_d"][:].rearrange("(t p) co -> p t co", p=128))
            emb_sb = p0.tile([128, CT], F32)
            nc.sync.dma_start(
                emb_sb[:], g["emb_d"][:].rearrange("(t p) -> p t", p=128))
            crow_ps = p0ps.tile([1, M], F32)
            for co2 in range(2):
                for t in range(CT):
                    nc.tensor.matmul(
                        crow_ps[0:1, co2 * 512:(co2 + 1) * 512],
                        emb_sb[:, t:t + 1],
                        wembt_sb[:, t, co2 * 512:(co2 + 1) * 512],
                        start=(t == 0), stop=(t == CT - 1))
            c_row = p0.tile([1, M], F32)
            nc.scalar.add(c_row[:], crow_ps[:], 1.0)
            nc.sync.dma_start(g["cdram"][:], c_row[:])
            nc.sync.dma_start(
                c_col[:],
                g["cdram"][:].rearrange("o (gg p) -> (o p) gg", p=128))

        # ============ x-norm ============
        with tc.tile_pool(name="p1", bufs=1) as p1:
            xs = p1.tile([128, CT, Lc], F32)
            nc.sync.dma_start(
                xs[:], g["xs_d"][:].rearrange("(t p) l -> p t l", p=128))
            sq = p1.tile([128, CT, Lc], F32)
            nc.vector.tensor_mul(sq[:], xs[:], xs[:])
            s01 = p1.tile([128, Lc], F32)
            nc.vector.tensor_add(s01[:], sq[:, 0, :], sq[:, 1, :])
            s23 = p1.tile([128, Lc], F32)
            nc.vector.tensor_add(s23[:], sq[:, 2, :], sq[:, 3, :])
            s_all = p1.tile([128, Lc], F32)
            nc.vector.tensor_add(s_all[:], s01[:], s23[:])
            ssq = p1.tile([128, Lc], F32)
            nc.gpsimd.partition_all_reduce(ssq[:], s_all[:], channels=128,
                                           reduce_op=RADD)
            nrm = p1.tile([128, Lc], F32)
            nc.scalar.activation(nrm[:], ssq[:], AF.Sqrt, scale=1.0 / C)
            nc.vector.tensor_scalar_add(nrm[:], nrm[:], EPS)
            inv = p1.tile([128, Lc], F32)
            nc.vector.reciprocal_approx_fast(inv[:], nrm[:])
            nc.vector.tensor_mul(
                xn[:], xs[:],
                inv[:, None, :].broadcast_to([128, CT, Lc]))

        # ============ MLP: res0 + depth conv + silu ============
        with tc.tile_pool(name="p2a", bufs=1) as p2a, \
             tc.tile_pool(name="p2aps", bufs=1, space="PSUM") as p2aps:
            w0t_sb = p2a.tile([128, CT, M], F32)
            nc.sync.dma_start(
                w0t_sb[:],
                g["w0t_d"][:].rearrange("(t p) co -> p t co", p=128))
            wdt_sb = p2a.tile([128, 72, 128], F32)
            nc.sync.dma_start(
                wdt_sb[:],
                g["wdt_d"][:].rearrange("p (s co) -> p s co", co=128))
            # pipeline: res0 for group gi+1 is emitted before the taps of
            # group gi, so the PE does not wait on the pad copies.
            y0ps_l = [None] * MT
            y0p_l = [None] * MT

            def emit_res0(gi):
                y0ps = p2aps.tile([128, Lc], F32, tag="y0ps", bufs=2)
                y0ps_l[gi] = y0ps
                for lc in range(2):
                    for t in range(CT):
                        nc.tensor.matmul(
                            y0ps[:, lc * 512:(lc + 1) * 512],
                            w0t_sb[:, t, gi * 128:(gi + 1) * 128],
                            xn[:, t, lc * 512:(lc + 1) * 512],
                            start=(t == 0), stop=(t == CT - 1))
                y0p = p2a.tile([128, 2, 520], F32, tag="y0p", bufs=2)
                y0p_l[gi] = y0p
                if gi < 2:
                    # zero the halos once per rotating buffer
                    nc.any.memset(y0p[:, :, 0:4], 0.0)
                    nc.any.memset(y0p[:, :, 516:520], 0.0)
                for row in range(2):
                    nc.vector.tensor_copy(
                        y0p[:, row, 4:516],
                        y0ps[:, row * 512:(row + 1) * 512])

            def emit_taps(gi):
                y0p = y0p_l[gi]
                y1ps = p2aps.tile([128, Lc], F32, tag="y1ps", bufs=2)
                for row in range(2):
                    for tap in range(9):
                        nc.tensor.matmul(
                            y1ps[:, row * 512:(row + 1) * 512],
                            wdt_sb[:, gi * 9 + tap, :],
                            y0p[:, row, tap:tap + 512],
                            start=(tap == 0), stop=(tap == 8))
                nc.scalar.activation(
                    y2[:, gi, :], y1ps[:], AF.Silu,
                    scale=c_col[:, gi:gi + 1])

            emit_res0(0)
            for gi in range(1, MT):
                emit_res0(gi)
                emit_taps(gi - 1)
            emit_taps(MT - 1)

        # ---- res1 + x_mid ----
        with tc.tile_pool(name="p2b", bufs=1) as p2b, \
             tc.tile_pool(name="p2bps", bufs=1, space="PSUM") as p2bps:
            w1t_sb = p2b.tile([128, MT, C], F32)
            nc.sync.dma_start(
                w1t_sb[:],
                g["w1t_d"][:].rearrange("(gg p) co -> p gg co", p=128))
            for lc in range(2):
                for co in range(CT):
                    y3ps = p2bps.tile([128, 512], F32, tag="y3ps", bufs=4)
                    for gi in range(MT):
                        nc.tensor.matmul(
                            y3ps[:],
                            w1t_sb[:, gi, co * 128:(co + 1) * 128],
                            y2[:, gi, lc * 512:(lc + 1) * 512],
                            start=(gi == 0), stop=(gi == MT - 1))
                    nc.vector.tensor_add(
                        xm[:, co, lc * 512:(lc + 1) * 512],
                        xn[:, co, lc * 512:(lc + 1) * 512],
                        y3ps[:])

    # ============ qkv ============
    with tc.tile_pool(name="p3", bufs=1) as p3, \
         tc.tile_pool(name="p3ps", bufs=1, space="PSUM") as p3ps:
        wk_sb = p3.tile([128, CT, 2, C], F32)
        nc.sync.dma_start(
            wk_sb[:], g["wk_d"][:].rearrange("(t p) e co -> p t e co", p=128))
        wq_sb = p3.tile([128, CT, 2, C], F32)
        nc.sync.dma_start(
            wq_sb[:], g["wq_d"][:].rearrange("(t p) e co -> p t e co", p=128))
        wvt_sb = p3.tile([128, CT, C], F32)
        nc.sync.dma_start(
            wvt_sb[:], g["wvt_d"][:].rearrange("(t p) co -> p t co", p=128))
        xpos = p3.tile([128, CT, Lc], F32)
        with tc.tile_pool(name="p3pos", bufs=1) as p3pos:
            pos = p3pos.tile([128, CT, Lc], F32)
            nc.sync.dma_start(
                pos[:], g["pos_d"][:].rearrange("(t p) l -> p t l", p=128))
            nc.vector.tensor_mul(xpos[:], xm[:], pos[:])

        def qk_proj(w_sb, raw, nrm):
            for h in range(HEADS):
                for lc in range(2):
                    ps = p3ps.tile([128, 512], F32, tag="qkps", bufs=4)
                    for t in range(CT):
                        nc.tensor.matmul(
                            ps[:], w_sb[:, t, 0, h * 128:(h + 1) * 128],
                            xm[:, t, lc * 512:(lc + 1) * 512],
                            start=(t == 0), stop=False)
                    for t in range(CT):
                        nc.tensor.matmul(
                            ps[:], w_sb[:, t, 1, h * 128:(h + 1) * 128],
                            xpos[:, t, lc * 512:(lc + 1) * 512],
                            start=False, stop=(t == CT - 1))
                    sqt = p3.tile([128, 512], F32, tag="sqt", bufs=2)
                    nc.scalar.square(sqt[:], ps[:])
                    nc.gpsimd.partition_all_reduce(
                        nrm[:, h, lc * 512:(lc + 1) * 512], sqt[:],
                        channels=128, reduce_op=RADD)
                    nc.vector.tensor_copy(
                        raw[:, h, lc * 512:(lc + 1) * 512], ps[:])

        def finish_norm(raw, nrm, invt):
            nc.scalar.activation(invt[:], nrm[:], AF.Sqrt, scale=1.0 / D)
            nc.vector.tensor_scalar_add(invt[:], invt[:], EPS)
            nc.vector.reciprocal_approx_fast(nrm[:], invt[:])
            nc.vector.tensor_mul(raw[:], raw[:], nrm[:])

        # ---- k, then AllGather k ----
        with tc.tile_pool(name="p3k", bufs=1) as p3k:
            kn = p3k.tile([128, HEADS, Lc], F32)
            nk = p3k.tile([128, HEADS, Lc], F32)
            ik = p3k.tile([128, HEADS, Lc], F32)
            qk_proj(wk_sb, kn, nk)
            finish_norm(kn, nk, ik)
            nc.sync.dma_start(
                g["agk_in"][:].rearrange("(h p) l -> p h l", p=128), kn[:])
            nc.gpsimd.collective_compute(
                "AllGather", mybir.AluOpType.bypass,
                replica_groups=[list(range(R))],
                ins=[g["agk_in"][:]], outs=[g["agk_out"][:]])

        # ---- v, then AllGather v ----
        with tc.tile_pool(name="p3v", bufs=1) as p3v:
            vraw = p3v.tile([128, 8, C], F32)
            nvs = p3v.tile([128, 8, HEADS], F32)
            ivv = p3v.tile([128, 8, HEADS], F32)
            for lt in range(8):
                vps = p3ps.tile([128, 512], F32, tag="qkps", bufs=4)
                for t in range(CT):
                    nc.tensor.matmul(
                        vps[:], xm[:, t, lt * 128:(lt + 1) * 128],
                        wvt_sb[:, t, :],
                        start=(t == 0), stop=(t == CT - 1))
                sqt = p3.tile([128, 512], F32, tag="sqt", bufs=2)
                nc.scalar.square(sqt[:], vps[:])
                nc.vector.tensor_reduce(
                    nvs[:, lt, :],
                    sqt[:].rearrange("p (h d) -> p h d", d=128),
                    axis=mybir.AxisListType.X, op=mybir.AluOpType.add)
                nc.vector.tensor_copy(vraw[:, lt, :], vps[:])
            nc.scalar.activation(ivv[:], nvs[:], AF.Sqrt, scale=1.0 / D)
            nc.vector.tensor_scalar_add(ivv[:], ivv[:], EPS)
            nc.vector.reciprocal_approx_fast(nvs[:], ivv[:])
            for lt in range(8):
                vv = vraw[:, lt, :].rearrange("p (h d) -> p h d", d=128)
                nc.vector.tensor_mul(
                    vv, vv,
                    nvs[:, lt, :, None].broadcast_to([128, HEADS, 128]))
            nc.sync.dma_start(
                g["agv_in"][:].rearrange("(lt p) co -> p lt co", p=128),
                vraw[:])
            nc.gpsimd.collective_compute(
                "AllGather", mybir.AluOpType.bypass,
                replica_groups=[list(range(R))],
                ins=[g["agv_in"][:]], outs=[g["agv_out"][:]])

        # ---- q (overlaps the collectives) ----
        with tc.tile_pool(name="p3q", bufs=1) as p3q:
            nq = p3q.tile([128, HEADS, Lc], F32)
            iq = p3q.tile([128, HEADS, Lc], F32)
            qk_proj(wq_sb, qn, nq)
            finish_norm(qn, nq, iq)
        nc.vector.tensor_scalar_mul(xh[:], xm[:], 0.5)


def _build_attention(nc, tc, pp, g, qn, xh):
    with tc.tile_pool(name="p4", bufs=1) as p4, \
         tc.tile_pool(name="p4ps", bufs=1, space="PSUM") as p4ps:
        NB = 8          # exp batches per quarter (2 ktiles per batch)
        NQ = 4          # quarters of the key range (16 ktiles each)
        for h in range(HEADS):
            ksb = p4.tile([128, KT * 128], F32, tag="ksb", bufs=2)
            nc.sync.dma_start(
                ksb[:].rearrange("p (r l) -> p r l", r=R),
                g["agk_out"][:, h, :, :].rearrange("r p l -> p r l"))
            vsb = p4.tile([128, KT, 128], F32, tag="vsb", bufs=1)
            nc.sync.dma_start(
                vsb[:],
                g["agv_out"][:, :, h * 128:(h + 1) * 128]
                .rearrange("t p d -> p t d"))
            for cch in range(2):
                q_c = qn[:, h, cch * 512:(cch + 1) * 512]
                pvps = p4ps.tile([128, 512], F32, tag="pvps", bufs=2)
                parts = p4.tile([128, NQ, Lc], F32, tag="parts", bufs=1)
                # software pipeline: emit scores+exp one batch ahead of pv
                # so the PE never waits on the Act engine's exp.
                pend = []

                def emit_pv(e_t, qu, b):
                    for j in range(2):
                        t = qu * 16 + b * 2 + j
                        nc.tensor.matmul(
                            pvps[:],
                            vsb[:, t, :],
                            e_t[:, b, j * 512:(j + 1) * 512],
                            start=(t == 0), stop=(t == KT - 1))

                for qu in range(NQ):
                    e_t = p4.tile([128, NB, Lc], F32, tag="e", bufs=1)
                    for b in range(NB):
                        t0 = qu * 16 + b * 2
                        sps = p4ps.tile([128, 1024], F32, tag="sps", bufs=3)
                        for j in range(2):
                            nc.tensor.matmul(
                                sps[:, j * 512:(j + 1) * 512],
                                ksb[:, (t0 + j) * 128:(t0 + j + 1) * 128],
                                q_c,
                                start=True, stop=True)
                        nc.scalar.activation(
                            e_t[:, b, :], sps[:], AF.Exp, scale=ISQ_D)
                        pend.append((e_t, qu, b))
                        if len(pend) > 1:
                            emit_pv(*pend.pop(0))
                    nc.vector.tensor_reduce(
                        parts[:, qu, :],
                        e_t[:].rearrange("p b l -> p l b"),
                        axis=mybir.AxisListType.X, op=mybir.AluOpType.add)
                while pend:
                    emit_pv(*pend.pop(0))
                s01 = p4.tile([128, Lc], F32, tag="s01", bufs=1)
                nc.vector.tensor_add(s01[:], parts[:, 0, :], parts[:, 1, :])
                s23 = p4.tile([128, Lc], F32, tag="s23", bufs=1)
                nc.vector.tensor_add(s23[:], parts[:, 2, :], parts[:, 3, :])
                nc.vector.tensor_add(s01[:], s01[:], s23[:])
                # fold the two ktile-parity halves; scale by sqrt(2)
                ssum = p4.tile([128, 512], F32, tag="ssum", bufs=1)
                nc.vector.tensor_add(ssum[:], s01[:, 0:512], s01[:, 512:1024])
                sred = p4.tile([128, 512], F32, tag="sred", bufs=1)
                nc.gpsimd.partition_all_reduce(sred[:], ssum[:], channels=128,
                                               reduce_op=RADD)
                nc.vector.tensor_scalar_mul(sred[:], sred[:], SQ2)
                isum = p4.tile([128, 512], F32, tag="isum", bufs=1)
                nc.vector.reciprocal_approx_fast(isum[:], sred[:])
                att = p4.tile([128, 512], F32, tag="att", bufs=2)
                nc.vector.tensor_mul(att[:], pvps[:], isum[:])
                nc.vector.tensor_add(
                    xh[:, h, cch * 512:(cch + 1) * 512],
                    xh[:, h, cch * 512:(cch + 1) * 512],
                    att[:])
        nc.vector.tensor_scalar(
            xh[:], xh[:], CLIP, -CLIP,
            op0=mybir.AluOpType.min, op1=mybir.AluOpType.max)
        nc.sync.dma_start(
            g["out_d"][:].rearrange("(t p) l -> p t l", p=128), xh[:])


def _host_prep(x, emb, pos_emb, emb_gain, w_res0, w_depth, w_emb, w_res1,
               w_qk, w_v):
    """Build shared weight arrays + per-core input shards."""
    f = np.float32
    w_res0 = np.asarray(w_res0, f).reshape(M, C)
    w_depth = np.asarray(w_depth, f).reshape(M, 128, 9)
    w_emb = np.asarray(w_emb, f).reshape(M, C)
    w_res1 = np.asarray(w_res1, f).reshape(C, M)
    w_qk = np.asarray(w_qk, f).reshape(2 * C, 2 * C)
    w_v = np.asarray(w_v, f).reshape(C, C)
    emb_gain = np.float32(emb_gain)

    w0t = np.ascontiguousarray((w_res0 * (1.0 / np.sqrt(C))).T)     # [C, M]
    wd = w_depth * (1.0 / np.sqrt(128 * 9))
    # wdt[p, (g*9+t)*128 + co] = wd[g*128+co, p, t]
    wdt = np.empty((128, 72 * 128), f)
    for gi in range(MT):
        blk = wd[gi * 128:(gi + 1) * 128]        # [co=128, ci=128, tap=9]
        wdt[:, gi * 9 * 128:(gi + 1) * 9 * 128] = (
            blk.transpose(1, 2, 0).reshape(128, 9 * 128))
    wembt = np.ascontiguousarray((w_emb * (emb_gain / np.sqrt(C))).T)  # [C, M]
    w1t = np.ascontiguousarray(
        (w_res1 * (1.0 / (SILU_SCALE * np.sqrt(M)))).T)             # [M, C]

    # qk rows: r = h*256 + dd*2 + s; cols: 2c (x part), 2c+1 (x*pos part)
    wqk = w_qk * (1.0 / np.sqrt(2 * C))
    wq3 = wqk.reshape(HEADS, 128, 2, 2 * C)   # [h, dd, s, ci2]
    wq_rows = wq3[:, :, 0, :].reshape(C, 2 * C)   # q rows [(h dd), ci2]
    wk_rows = wq3[:, :, 1, :].reshape(C, 2 * C)
    wq = np.empty((C, 2, C), f)
    wq[:, 0, :] = (wq_rows[:, 0::2] * ISQ2).T     # even ci: xm part
    wq[:, 1, :] = wq_rows[:, 1::2].T              # odd ci: xm*pos part
    wk = np.empty((C, 2, C), f)
    wk[:, 0, :] = (wk_rows[:, 0::2] * ISQ2).T
    wk[:, 1, :] = wk_rows[:, 1::2].T
    wvt = np.ascontiguousarray((w_v * (1.0 / np.sqrt(C) * ISQ2)).T)  # [C, C]

    x = np.asarray(x, f).reshape(C, 16, 512)
    pos = np.asarray(pos_emb, f).reshape(C, 16, 512) * ISQ2
    embv = np.ascontiguousarray(np.asarray(emb, f).reshape(C))

    shared = dict(
        embv=embv, w0t=w0t, wdt=wdt, wembt=wembt, w1t=w1t,
        wq=wq, wk=wk, wvt=wvt)
    in_maps = []
    for r in range(R):
        m = dict(shared)
        m["xs"] = np.ascontiguousarray(
            x[:, 2 * r:2 * r + 2, :].reshape(C, Lc))
        m["pos"] = np.ascontiguousarray(
            pos[:, 2 * r:2 * r + 2, :].reshape(C, Lc))
        in_maps.append(m)
    return in_maps


def kernel(**inputs):
    if "nc" not in _CACHE:
        _CACHE["nc"] = _build_nc()
    nc = _CACHE["nc"]
    in_maps = _host_prep(
        inputs["x"], inputs["emb"], inputs["pos_emb"], inputs["emb_gain"],
        inputs["w_res0"], inputs["w_depth"], inputs["w_emb"],
        inputs["w_res1"], inputs["w_qk"], inputs["w_v"])
    res = run_bass_kernel_spmd(nc, in_maps, list(range(R)))
    out = np.empty((1, C, 16, 512), np.float32)
    for r in range(R):
        out[0, :, 2 * r:2 * r + 2, :] = res.results[r]["out"].reshape(C, 2, 512)
    return out
